# revision 51
# baseline (speedup 1.0000x reference)
"""Causal self-attention with RoPE on 8 Trainium2 NeuronCores.

Sharding: tensor-parallel over heads (2 heads/core) through QKV projection,
RoPE and attention; AllToAll reshards attention output from head-split to
token-split; out-projection is token-parallel with full out_w per core
(no reduction needed). Output: each core produces its 512-token slice.

The kernel is PE-sequencer/instruction-bound (~240 ns issue cost per matmul),
so the design minimizes matmul instruction count and keeps the PE free of
softmax-denominator streaming:
  phase 1: fused q/k/v projection, 6 f-tiles x 16 kt x 4 blocks of N=1024
           (384 matmuls); v is produced feature-major and transposed on the
           PE (64 [128,128] transposes) into token-major v_sb for PV.
  phase 2: per (head, batch, 1024-query half): one scores matmul per
           key-tile (N clipped at the causal diagonal), exp on ACT, one PV
           matmul per key-tile. The softmax denominator is accumulated
           elementwise in bf16: the first 512 queries on the Pool engine,
           the second 512 on DVE (both otherwise idle), then reduced across
           partitions by two ones-matmuls into a PSUM tile borrowed from
           the scores pool ([128,1024] output = partition-broadcast sums).
  phase 4: out-projection with N=2048 (64 matmuls), two passes (even kt from
           the first AllToAll, odd kt from the second) so each AllToAll
           hides behind compute; out-proj weights prefetch on the Pool DMA
           queue during earlier phases.
"""
import math
import numpy as np
import ml_dtypes

import concourse.bass as bass
import concourse.mybir as mybir
import concourse.tile as tile
from concourse import bacc
from concourse.bass_utils import run_bass_kernel_spmd

F32 = mybir.dt.float32
F32R = mybir.dt.float32r
BF16 = mybir.dt.bfloat16
AF = mybir.ActivationFunctionType
ALU = mybir.AluOpType

N_CORES = 8


def legalize_waits(nc, max_waits=1):
    """This walrus build only encodes one sync-wait per TPB instruction.
    Move extra waits emitted by Tile onto same-engine NoOps inserted
    immediately before the instruction."""
    n_split = 0
    for fn in nc.m.functions:
        for bb in fn.blocks:
            new_insts = []
            for inst in bb.instructions:
                si = getattr(inst, "sync_info", None)
                waits = list(si.on_wait) if si is not None and si.on_wait else []
                if len(waits) > max_waits and type(inst).__name__ != "InstNoOp":
                    extra, keep = waits[:-max_waits], waits[-max_waits:]
                    for k, w in enumerate(extra):
                        nop = mybir.InstNoOp(
                            name=f"{inst.name}_waitnop{k}",
                            engine=inst.engine,
                            ins=[],
                            outs=[],
                            sync_info=mybir.SyncInfo(on_wait=[w], on_update=[]),
                        )
                        nc.register_instruction(nop)
                        new_insts.append(nop)
                    inst.sync_info = mybir.SyncInfo(
                        on_wait=keep, on_update=list(si.on_update)
                    )
                    n_split += 1
                new_insts.append(inst)
            bb.instructions = new_insts
    return n_split


def build_nc(B=2, T=2048, D=2048, H=16, fake_cc=False, n_loop=1, dummy_io=False):
    HD = D // H                  # 128, head dim
    NT = B * T                   # total tokens
    HPC = H // N_CORES           # heads per core (2)
    DC = HPC * HD                # head channels per core (256)
    KT = D // 128                # contraction tiles for projections (16)
    NB = NT // 1024              # 1024-token projection blocks (4)
    GH = T // 1024               # 1024-query halves per batch element (2)
    S = NT // N_CORES            # AllToAll shard = tokens per core (512)
    SCALE = 1.0 / math.sqrt(HD)

    nc = bacc.Bacc("TRN2", target_bir_lowering=False, debug=False, num_devices=N_CORES)
    # dummy_io: declare data tensors as internal DRAM (uninitialized) so the
    # timing NEFF has no big inputs to ship through the axon tunnel.
    ik = {"kind": "ExternalInput"} if not dummy_io else {}
    xT_e = nc.dram_tensor("xT", [D, NT], BF16, **ik)
    wqkv_e = nc.dram_tensor("wqkv", [D, 6 * HD], BF16, **ik)
    bqkv_e = nc.dram_tensor("bqkv", [6 * HD], F32, **ik)
    cos_e = nc.dram_tensor("cosT", [HD, NT], BF16, **ik)
    sin_e = nc.dram_tensor("sinT", [HD, NT], BF16, **ik)
    masks_e = nc.dram_tensor("masks", [128, 128], BF16, **ik)
    negm_e = nc.dram_tensor("negm", [128, 128], F32, **ik)
    ident_e = nc.dram_tensor("ident", [128, 128], BF16, **ik)
    owT_e = nc.dram_tensor("owT", [D, D], BF16, **ik)
    ob_e = nc.dram_tensor("ob", [D], F32, **ik)
    out_e = nc.dram_tensor("out", [S, D], F32, kind="ExternalOutput")

    with tile.TileContext(nc) as tc:
      for _it in range(n_loop):
        with tc.tile_pool(name=f"persist{_it}", bufs=1) as pp, \
             tc.tile_pool(name=f"dram{_it}", bufs=1, space="DRAM") as dp:
            owr = owT_e.rearrange("(kt p) f -> p kt f", p=128)

            # ---- persistent small tiles ----
            masks = pp.tile([128, 128], BF16, tag="masks", name="masks")
            nc.sync.dma_start(masks[:], masks_e[:])
            negm = pp.tile([128, 128], F32, tag="negm", name="negm")
            nc.sync.dma_start(negm[:], negm_e[:])
            ident = pp.tile([128, 128], BF16, tag="ident", name="ident")
            nc.sync.dma_start(ident[:], ident_e[:])
            bqkv = pp.tile([128, 6], F32, tag="bqkv", name="bqkv")
            nc.sync.dma_start(bqkv[:], bqkv_e.rearrange("(m p) -> p m", p=128))
            obB = pp.tile([128, D], F32, tag="obB", name="obB")
            nc.sync.dma_start(obB[:1, :], ob_e[None, :])
            nc.gpsimd.partition_broadcast(obB[:], obB[:1, :])
            onesB = pp.tile([128, 128], BF16, tag="onesB", name="onesB")
            nc.vector.memset(onesB[:], 1.0)

            Zs = [dp.tile([N_CORES, HD, S], BF16, tag=f"Z{i}", name=f"Z{i}")
                  for i in range(HPC)]
            ZGs = [dp.tile([N_CORES, HD, S], BF16, tag=f"ZG{i}", name=f"ZG{i}")
                   for i in range(HPC)]

            if True:
                # q/k feature-major [head-dim, tokens]; v token-major
                qk = [pp.tile([128, NT], BF16, tag=f"qk{m}", name=f"qk{m}")
                      for m in range(4)]
                v_sb = pp.tile([128, NT // 128, DC], BF16, tag="v", name="v")

                # ---- phase 1: fused QKV projection + RoPE + v transpose ----
                with tc.tile_pool(name=f"p1w{_it}", bufs=1) as wp, \
                     tc.tile_pool(name=f"p1x{_it}", bufs=2) as xp, \
                     tc.tile_pool(name=f"p1t{_it}", bufs=3) as tp, \
                     tc.tile_pool(name=f"p1ps{_it}", bufs=2, space="PSUM") as ps, \
                     tc.tile_pool(name=f"p1pt{_it}", bufs=2, space="PSUM") as pst:
                    # PE warm-up: junk transposes keep the PE busy (and the
                    # clock ramped) while the first x/weight DMAs land
                    for wu in range(144):
                        ptw = pst.tile([128, 512], BF16, tag="pt", name="ptw")
                        nc.tensor.transpose(
                            ptw[:, bass.ts(wu % 4, 128)], onesB[:], onesB[:])
                    # first-needed data first: x block 0, then qkv weights
                    xTr = xT_e.rearrange("(kt p) t -> p kt t", p=128)
                    xb0 = xp.tile([128, KT, 1024], BF16, tag="xb", name="xb")
                    nc.sync.dma_start(xb0[:, :KT // 2, :], xTr[:, :KT // 2, bass.ts(0, 1024)])
                    nc.sync.dma_start(xb0[:, KT // 2:, :], xTr[:, KT // 2:, bass.ts(0, 1024)])
                    wqkv_sb = wp.tile([128, KT, 6 * HD], BF16, tag="wqkv", name="wqkv")
                    wqr = wqkv_e.rearrange("(kt p) f -> p kt f", p=128)
                    nc.scalar.dma_start(wqkv_sb[:, :KT // 2, :], wqr[:, :KT // 2, :])
                    nc.scalar.dma_start(wqkv_sb[:, KT // 2:, :], wqr[:, KT // 2:, :])
                    cos_sb = wp.tile([128, NT], BF16, tag="cos", name="cos")
                    nc.scalar.dma_start(cos_sb[:], cos_e[:])
                    sin_sb = wp.tile([128, NT], BF16, tag="sin", name="sin")
                    nc.scalar.dma_start(sin_sb[:], sin_e[:])
                    for blk in range(NB):
                        tsl = bass.ts(blk, 1024)
                        if blk == 0:
                            xb = xb0
                        else:
                            xb = xp.tile([128, KT, 1024], BF16, tag="xb", name="xb")
                            nc.sync.dma_start(xb[:, :KT // 2, :], xTr[:, :KT // 2, tsl])
                            nc.sync.dma_start(xb[:, KT // 2:, :], xTr[:, KT // 2:, tsl])
                        for m in ([0, 2, 4, 1, 3, 5] if blk == NB - 1 else range(6)):
                            psqk = ps.tile([128, 1024], F32, tag="ps", name="ps")
                            for kt in range(KT):
                                for mh in range(2):
                                    nc.tensor.matmul(
                                        psqk[:, bass.ts(mh, 512)],
                                        wqkv_sb[:, kt, bass.ts(m, 128)],
                                        xb[:, kt, bass.ts(mh, 512)],
                                        start=(kt == 0),
                                        stop=(kt == KT - 1),
                                        skip_group_check=True,
                                    )
                            if m < 4:
                                # q/k: evict with bias, RoPE in place
                                nc.scalar.activation(
                                    qk[m][:, tsl], psqk[:], AF.Identity,
                                    bias=bqkv[:, m:m + 1], scale=1.0,
                                )
                                qm = qk[m][:, tsl]
                                qsw = tp.tile([128, 1024], BF16, tag="qsw", name="qsw")
                                nc.sync.dma_start(qsw[0:64, :], qm[64:128, :])
                                nc.sync.dma_start(qsw[64:128, :], qm[0:64, :])
                                nc.vector.tensor_mul(qsw[:], qsw[:], sin_sb[:, tsl])
                                nc.vector.tensor_mul(qm, qm, cos_sb[:, tsl])
                                nc.vector.tensor_add(qm, qm, qsw[:])
                            else:
                                # v: evict with bias feature-major, then
                                # transpose 128x128 token tiles into v_sb
                                hh = m - 4
                                vT = tp.tile([128, 1024], BF16, tag="qsw", name="vT")
                                nc.scalar.activation(
                                    vT[:], psqk[:], AF.Identity,
                                    bias=bqkv[:, m:m + 1], scale=1.0,
                                )
                                for half in range(2):
                                    pt4 = pst.tile([128, 512], BF16, tag="pt", name="pt")
                                    for tt in range(4):
                                        nc.tensor.transpose(
                                            pt4[:, bass.ts(tt, 128)],
                                            vT[:, bass.ds(half * 512 + tt * 128, 128)],
                                            ident[:],
                                        )
                                    g0 = blk * 8 + half * 4
                                    nc.vector.tensor_copy(
                                        v_sb[:, g0:g0 + 4, bass.ts(hh, HD)],
                                        pt4[:].rearrange("p (a b) -> p a b", a=4),
                                    )

                zg_sb = [pp.tile([128, N_CORES, S], BF16, tag=f"zg{i}", name=f"zg{i}")
                         for i in range(HPC)]

                # out-proj weights live in SBUF freed by the phase-1 pools;
                # first half prefetches on the idle Pool DMA queue during
                # attention, second half after the first AllToAll
                owp_cm = tc.tile_pool(name=f"ow{_it}", bufs=1)
                owp = owp_cm.__enter__()
                ows = [owp.tile([128, KT // 2, D], BF16, tag=f"ow{pa}", name=f"ow{pa}")
                       for pa in range(2)]
                ow_sel = owr[:, 0::2, :]
                for c4 in range(8):
                    nc.gpsimd.dma_start(
                        ows[0][:, c4, :], ow_sel[:, c4, :])

                # ---- phase 2: attention per (head, batch, 512-row block) ----
                # 1024-wide score chunks (2 key-tiles per exp) amortize ACT
                # overhead; scores/exp/PV/denominator are clipped at the
                # causal diagonal (only queries >= 128*m computed for
                # diagonal key-tile m). The softmax denominator accumulates
                # on the PE via ones-matmuls whose M=128 output arrives
                # partition-broadcast, so no broadcast op is needed before
                # the reciprocal. Keeping the denominator on the PE keeps
                # the PE the pacing engine (no cross-engine stalls, pstate
                # stays at full clock).
                with tc.tile_pool(name=f"p2t{_it}", bufs=6) as tp2, \
                     tc.tile_pool(name=f"p2o{_it}", bufs=2) as op2, \
                     tc.tile_pool(name=f"p2r{_it}", bufs=2) as rp2, \
                     tc.tile_pool(name=f"p2ps{_it}", bufs=3, space="PSUM") as ps2, \
                     tc.tile_pool(name=f"p2po{_it}", bufs=1, space="PSUM") as ps2o, \
                     tc.tile_pool(name=f"p2pd{_it}", bufs=1, space="PSUM") as ps2d:
                    for hh in range(HPC):
                        for b in range(B):
                            qT = qk[hh]
                            kTt = qk[2 + hh]
                            for rb in range(T // 512):
                                qbase = b * T + rb * 512
                                pso = ps2o.tile([128, 512], F32, tag="pso", name="pso")
                                psd = ps2d.tile([128, 512], F32, tag="psd", name="psd")
                                njt = 4 * rb + 4
                                for jc in range(njt // 2):
                                    pss = ps2.tile([128, 1024], F32, tag="pss", name="pss")
                                    offs = []
                                    for half in range(2):
                                        jt = 2 * jc + half
                                        m = jt - 4 * rb
                                        off = 128 * m if m >= 0 else 0
                                        offs.append((jt, off))
                                        nc.tensor.matmul(
                                            pss[:, bass.ds(512 * half + off, 512 - off)],
                                            kTt[:, bass.ds(b * T + jt * 128, 128)],
                                            qT[:, bass.ds(qbase + off, 512 - off)],
                                            start=True, stop=True,
                                        )
                                    pT = tp2.tile([128, 1024], BF16, tag="pT", name="pT")
                                    if offs[1][1] == 0:
                                        nc.scalar.activation(
                                            pT[:], pss[:], AF.Exp, scale=SCALE)
                                    else:
                                        for half in range(2):
                                            jt, off = offs[half]
                                            sl = bass.ds(512 * half + off, 512 - off)
                                            nc.scalar.activation(
                                                pT[:, sl], pss[:, sl], AF.Exp,
                                                scale=SCALE)
                                            if jt >= 4 * rb:
                                                tri = bass.ds(512 * half + off, 128)
                                                nc.vector.tensor_mul(
                                                    pT[:, tri], pT[:, tri], masks[:])
                                    for half in range(2):
                                        jt, off = offs[half]
                                        psl = bass.ds(512 * half + off, 512 - off)
                                        osl = bass.ds(off, 512 - off)
                                        nc.tensor.matmul(
                                            psd[:, osl], onesB[:], pT[:, psl],
                                            start=(jt == 0), stop=(jt == njt - 1),
                                            skip_group_check=True,
                                        )
                                        nc.tensor.matmul(
                                            pso[:, osl],
                                            v_sb[:, (b * T) // 128 + jt, bass.ts(hh, HD)],
                                            pT[:, psl],
                                            start=(jt == 0),
                                            stop=(jt == njt - 1),
                                            skip_group_check=True,
                                        )
                                recipB = rp2.tile([128, 512], F32, tag="recipB",
                                                  name="recipB")
                                nc.vector.reciprocal(recipB[:], psd[:])
                                oT = op2.tile([128, 512], BF16, tag="oT", name="oT")
                                nc.vector.tensor_mul(oT[:], pso[:], recipB[:])
                                sh = qbase // S
                                nc.sync.dma_start(Zs[hh][sh, :, :], oT[:])
                        # reshard this head-half while the next one computes
                        if fake_cc:
                            nc.sync.dma_start(ZGs[hh][:], Zs[hh][:])
                        else:
                            nc.gpsimd.collective_compute(
                                "AllToAll", ALU.bypass,
                                replica_groups=[list(range(N_CORES))],
                                ins=[Zs[hh][:]], outs=[ZGs[hh][:]],
                            )
                        # pull this half's out-proj operand into SBUF on the
                        # Pool queue (so its collective-wait never blocks the
                        # SP store queue), chunked so out-proj can start on
                        # the first chunk
                        for c8 in range(N_CORES):
                            nc.gpsimd.dma_start(
                                zg_sb[hh][:, c8, :], ZGs[hh][c8, :, :])
                        if hh == 0:
                            # prefetch second half of out-proj weights now
                            ow_sel = owr[:, 1::2, :]
                            for c4 in range(8):
                                nc.gpsimd.dma_start(
                                    ows[1][:, c4, :], ow_sel[:, c4, :])

            # ---- phase 4: out projection on own token slice ----
            # Two-pass contraction: all zg0 (first AllToAll) partial sums are
            # computed and evicted before any zg1 tile is touched, so the
            # second AllToAll and the zg1 load hide behind real matmul work.
            with tc.tile_pool(name=f"p4z{_it}", bufs=1) as zp, \
                 tc.tile_pool(name=f"p4t{_it}", bufs=4) as tp4, \
                 tc.tile_pool(name=f"p4ps{_it}", bufs=2, space="PSUM") as ps4:
                accbig = zp.tile([128, S // 128, D], F32, tag="accbig", name="accbig")
                # keep the PE busy (clock ramped) while the first zg chunks
                # arrive: harmless matmuls on resident attention tiles
                psj = ps4.tile([128, D], F32, tag="ps4", name="psj")
                for ju in range(12):
                    nc.tensor.matmul(
                        psj[:, bass.ts(ju % 4, 512)],
                        qk[0][:, bass.ts(ju, 128)], qk[2][:, bass.ts(0, 512)],
                        start=True, stop=True, skip_group_check=True,
                    )
                for pa in range(2):
                    if pa == 1:
                        # filler while the second AllToAll lands
                        psj2 = ps4.tile([128, D], F32, tag="ps4", name="psj2")
                        for ju in range(40):
                            nc.tensor.matmul(
                                psj2[:, bass.ts(ju % 4, 512)],
                                zg_sb[0][:, ju % 8, bass.ts(0, 128)],
                                ows[0][:, ju % 8, bass.ts(ju % 4, 512)],
                                start=True, stop=True, skip_group_check=True,
                            )
                    for tt in range(S // 128):
                        pso4 = ps4.tile([128, D], F32, tag="ps4", name="ps4")
                        for zt in range(KT // HPC):
                            for eh in range(4):
                                nc.tensor.matmul(
                                    pso4[:, bass.ts(eh, 512)],
                                    zg_sb[pa][:, zt, bass.ts(tt, 128)],
                                    ows[pa][:, zt, bass.ts(eh, 512)],
                                    start=(zt == 0),
                                    stop=(zt == KT // HPC - 1),
                                    skip_group_check=True,
                                )
                        if pa == 0:
                            # fold the output bias in here so pass 1 needs a
                            # single add before each store
                            nc.vector.tensor_add(accbig[:, tt, :], pso4[:], obB[:])
                        else:
                            # finer-grained eviction+store pipeline to shrink
                            # the end-of-kernel DMA tail
                            for e in range(4):
                                esl = bass.ts(e, 512)
                                of = tp4.tile([128, 512], F32, tag="of", name="of")
                                nc.vector.tensor_add(
                                    of[:], pso4[:, esl], accbig[:, tt, esl])
                                nc.sync.dma_start(out_e[bass.ts(tt, 128), esl], of[:])
            owp_cm.__exit__(None, None, None)

    nc.compile()          # Bacc pass pipeline (library loads, nop fusion, regs)
    legalize_waits(nc)    # must run after all nop-fusion passes
    bass.Bass.finalize(nc)  # freeze without re-running Bacc compile
    return nc


def _prep_inputs(x, rope_cos, rope_sin, qkv_w, qkv_b, out_w, out_b, B, T, D, H):
    HD = D // H
    NT = B * T
    HPC = H // N_CORES
    bf = ml_dtypes.bfloat16

    x2 = np.ascontiguousarray(x.reshape(NT, D).T).astype(bf)           # [D, NT]
    cosT = np.ascontiguousarray(
        np.tile(rope_cos[0, 0].T, (1, B))).astype(bf)                   # [HD, NT]
    s2 = np.tile(rope_sin[0, 0].T, (1, B)).copy()
    s2[:HD // 2] *= -1.0
    sinT = np.ascontiguousarray(s2).astype(bf)
    owT = np.ascontiguousarray(out_w.T).astype(bf)                      # [D, D]
    ob = out_b.astype(np.float32)

    c_grid = np.arange(128)[None, :]
    p_grid = np.arange(128)[:, None]
    masks = (c_grid >= p_grid).astype(bf)                               # [128,128]
    negm = np.where(c_grid >= p_grid, 0.0, -1e5).astype(np.float32)     # [128,128]
    ident = np.eye(128).astype(bf)

    in_maps = []
    for c in range(N_CORES):
        heads = [HPC * c + i for i in range(HPC)]
        q_rows = np.concatenate([qkv_w[h * HD:(h + 1) * HD] for h in heads])
        k_rows = np.concatenate([qkv_w[D + h * HD:D + (h + 1) * HD] for h in heads])
        v_rows = np.concatenate([qkv_w[2 * D + h * HD:2 * D + (h + 1) * HD] for h in heads])
        wqkv = np.ascontiguousarray(
            np.concatenate([q_rows, k_rows, v_rows]).T).astype(bf)      # [D, 768]
        bq = np.concatenate([qkv_b[h * HD:(h + 1) * HD] for h in heads])
        bk = np.concatenate([qkv_b[D + h * HD:D + (h + 1) * HD] for h in heads])
        bv = np.concatenate([qkv_b[2 * D + h * HD:2 * D + (h + 1) * HD] for h in heads])
        bqkv = np.concatenate([bq, bk, bv]).astype(np.float32)          # [768]
        in_maps.append({
            "xT": x2, "wqkv": wqkv, "bqkv": bqkv,
            "cosT": cosT, "sinT": sinT, "masks": masks, "negm": negm,
            "ident": ident,
            "owT": owT, "ob": ob,
        })
    return in_maps


_NC_CACHE = {}


def kernel(x, rope_cos, rope_sin, qkv_w, qkv_b, out_w, out_b):
    B, T, D = x.shape
    H = 16
    NT = B * T
    S = NT // N_CORES
    key = (B, T, D, H)
    if key not in _NC_CACHE:
        _NC_CACHE[key] = build_nc(B, T, D, H)
    nc = _NC_CACHE[key]
    in_maps = _prep_inputs(
        np.asarray(x), np.asarray(rope_cos), np.asarray(rope_sin),
        np.asarray(qkv_w), np.asarray(qkv_b), np.asarray(out_w),
        np.asarray(out_b), B, T, D, H,
    )
    res = run_bass_kernel_spmd(nc, in_maps, core_ids=list(range(N_CORES)))
    out = np.empty((NT, D), np.float32)
    for c in range(N_CORES):
        out[c * S:(c + 1) * S] = res.results[c]["out"]
    return out.reshape(B, T, D)


# revision 54
# speedup vs baseline: 1.1312x; 1.1312x over previous
"""Causal self-attention with RoPE on 8 Trainium2 NeuronCores.

Sharding: tensor-parallel over heads (2 heads/core) through QKV projection,
RoPE and attention; AllToAll reshards attention output from head-split to
token-split; out-projection is token-parallel with full out_w per core
(no reduction needed). Output: each core produces its 512-token slice.

The kernel is PE-sequencer/instruction-bound (~240 ns issue cost per matmul),
so the design minimizes matmul instruction count and keeps the PE free of
softmax-denominator streaming:
  phase 1: fused q/k/v projection, 6 f-tiles x 16 kt x 4 blocks of N=1024
           (384 matmuls); v is produced feature-major and transposed on the
           PE (64 [128,128] transposes) into token-major v_sb for PV.
  phase 2: per (head, batch, 1024-query half): one scores matmul per
           key-tile (N clipped at the causal diagonal), exp on ACT, one PV
           matmul per key-tile. The softmax denominator is accumulated
           elementwise in bf16: the first 512 queries on the Pool engine,
           the second 512 on DVE (both otherwise idle), then reduced across
           partitions by two ones-matmuls into a PSUM tile borrowed from
           the scores pool ([128,1024] output = partition-broadcast sums).
  phase 4: out-projection with N=2048 (64 matmuls), two passes (even kt from
           the first AllToAll, odd kt from the second) so each AllToAll
           hides behind compute; out-proj weights prefetch on the Pool DMA
           queue during earlier phases.
"""
import math
import numpy as np
import ml_dtypes

import concourse.bass as bass
import concourse.mybir as mybir
import concourse.tile as tile
from concourse import bacc
from concourse.bass_utils import run_bass_kernel_spmd

F32 = mybir.dt.float32
F32R = mybir.dt.float32r
BF16 = mybir.dt.bfloat16
AF = mybir.ActivationFunctionType
ALU = mybir.AluOpType

N_CORES = 8


def legalize_waits(nc, max_waits=1):
    """This walrus build only encodes one sync-wait per TPB instruction.
    Move extra waits emitted by Tile onto same-engine NoOps inserted
    immediately before the instruction."""
    n_split = 0
    for fn in nc.m.functions:
        for bb in fn.blocks:
            new_insts = []
            for inst in bb.instructions:
                si = getattr(inst, "sync_info", None)
                waits = list(si.on_wait) if si is not None and si.on_wait else []
                if len(waits) > max_waits and type(inst).__name__ != "InstNoOp":
                    extra, keep = waits[:-max_waits], waits[-max_waits:]
                    for k, w in enumerate(extra):
                        nop = mybir.InstNoOp(
                            name=f"{inst.name}_waitnop{k}",
                            engine=inst.engine,
                            ins=[],
                            outs=[],
                            sync_info=mybir.SyncInfo(on_wait=[w], on_update=[]),
                        )
                        nc.register_instruction(nop)
                        new_insts.append(nop)
                    inst.sync_info = mybir.SyncInfo(
                        on_wait=keep, on_update=list(si.on_update)
                    )
                    n_split += 1
                new_insts.append(inst)
            bb.instructions = new_insts
    return n_split


def build_nc(B=2, T=2048, D=2048, H=16, fake_cc=False, n_loop=1, dummy_io=False):
    HD = D // H                  # 128, head dim
    NT = B * T                   # total tokens
    HPC = H // N_CORES           # heads per core (2)
    DC = HPC * HD                # head channels per core (256)
    KT = D // 128                # contraction tiles for projections (16)
    NB = NT // 1024              # 1024-token projection blocks (4)
    GH = T // 1024               # 1024-query halves per batch element (2)
    S = NT // N_CORES            # AllToAll shard = tokens per core (512)
    SCALE = 1.0 / math.sqrt(HD)

    nc = bacc.Bacc("TRN2", target_bir_lowering=False, debug=False, num_devices=N_CORES)
    # dummy_io: declare data tensors as internal DRAM (uninitialized) so the
    # timing NEFF has no big inputs to ship through the axon tunnel.
    ik = {"kind": "ExternalInput"} if not dummy_io else {}
    xT_e = nc.dram_tensor("xT", [D, NT], BF16, **ik)
    wqkv_e = nc.dram_tensor("wqkv", [D, 6 * HD], BF16, **ik)
    bqkv_e = nc.dram_tensor("bqkv", [6 * HD], F32, **ik)
    cos_e = nc.dram_tensor("cosT", [HD, NT], BF16, **ik)
    sin_e = nc.dram_tensor("sinT", [HD, NT], BF16, **ik)
    masks_e = nc.dram_tensor("masks", [128, 128], BF16, **ik)
    negm_e = nc.dram_tensor("negm", [128, 128], F32, **ik)
    ident_e = nc.dram_tensor("ident", [128, 128], BF16, **ik)
    owT_e = nc.dram_tensor("owT", [D, D], BF16, **ik)
    ob_e = nc.dram_tensor("ob", [D], F32, **ik)
    out_e = nc.dram_tensor("out", [S, D], F32, kind="ExternalOutput")

    with tile.TileContext(nc) as tc:
      for _it in range(n_loop):
        with tc.tile_pool(name=f"persist{_it}", bufs=1) as pp, \
             tc.tile_pool(name=f"dram{_it}", bufs=1, space="DRAM") as dp:
            owr = owT_e.rearrange("(kt p) f -> p kt f", p=128)

            # ---- persistent small tiles ----
            masks = pp.tile([128, 128], BF16, tag="masks", name="masks")
            nc.sync.dma_start(masks[:], masks_e[:])
            negm = pp.tile([128, 128], F32, tag="negm", name="negm")
            nc.sync.dma_start(negm[:], negm_e[:])
            ident = pp.tile([128, 128], BF16, tag="ident", name="ident")
            nc.sync.dma_start(ident[:], ident_e[:])
            bqkv = pp.tile([128, 6], F32, tag="bqkv", name="bqkv")
            nc.sync.dma_start(bqkv[:], bqkv_e.rearrange("(m p) -> p m", p=128))
            obB = pp.tile([128, D], F32, tag="obB", name="obB")
            nc.sync.dma_start(obB[:1, :], ob_e[None, :])
            nc.gpsimd.partition_broadcast(obB[:], obB[:1, :])
            onesB = pp.tile([128, 128], BF16, tag="onesB", name="onesB")
            nc.vector.memset(onesB[:], 1.0)

            Zs = [dp.tile([N_CORES, HD, S], BF16, tag=f"Z{i}", name=f"Z{i}")
                  for i in range(HPC)]
            ZGs = [dp.tile([N_CORES, HD, S], BF16, tag=f"ZG{i}", name=f"ZG{i}")
                   for i in range(HPC)]

            if True:
                # q/k feature-major [head-dim, tokens]; v token-major
                qk = [pp.tile([128, NT], BF16, tag=f"qk{m}", name=f"qk{m}")
                      for m in range(4)]
                v_sb = pp.tile([128, NT // 128, DC], BF16, tag="v", name="v")

                # ---- phase 1: fused QKV projection + RoPE + v transpose ----
                with tc.tile_pool(name=f"p1w{_it}", bufs=1) as wp, \
                     tc.tile_pool(name=f"p1x{_it}", bufs=2) as xp, \
                     tc.tile_pool(name=f"p1t{_it}", bufs=3) as tp, \
                     tc.tile_pool(name=f"p1ps{_it}", bufs=2, space="PSUM") as ps, \
                     tc.tile_pool(name=f"p1pt{_it}", bufs=2, space="PSUM") as pst:
                    # PE warm-up: junk transposes keep the PE busy (and the
                    # clock ramped) while the first x/weight DMAs land
                    for wu in range(144):
                        ptw = pst.tile([128, 512], BF16, tag="pt", name="ptw")
                        nc.tensor.transpose(
                            ptw[:, bass.ts(wu % 4, 128)], onesB[:], onesB[:])
                    # first-needed data first: x block 0, then qkv weights
                    xTr = xT_e.rearrange("(kt p) t -> p kt t", p=128)
                    xb0 = xp.tile([128, KT, 1024], BF16, tag="xb", name="xb")
                    nc.sync.dma_start(xb0[:, :KT // 2, :], xTr[:, :KT // 2, bass.ts(0, 1024)])
                    nc.sync.dma_start(xb0[:, KT // 2:, :], xTr[:, KT // 2:, bass.ts(0, 1024)])
                    wqkv_sb = wp.tile([128, KT, 6 * HD], BF16, tag="wqkv", name="wqkv")
                    wqr = wqkv_e.rearrange("(kt p) f -> p kt f", p=128)
                    nc.scalar.dma_start(wqkv_sb[:, :KT // 2, :], wqr[:, :KT // 2, :])
                    nc.scalar.dma_start(wqkv_sb[:, KT // 2:, :], wqr[:, KT // 2:, :])
                    cos_sb = wp.tile([128, NT], BF16, tag="cos", name="cos")
                    nc.scalar.dma_start(cos_sb[:], cos_e[:])
                    sin_sb = wp.tile([128, NT], BF16, tag="sin", name="sin")
                    nc.scalar.dma_start(sin_sb[:], sin_e[:])
                    for blk in range(NB):
                        tsl = bass.ts(blk, 1024)
                        if blk == 0:
                            xb = xb0
                        else:
                            xb = xp.tile([128, KT, 1024], BF16, tag="xb", name="xb")
                            nc.sync.dma_start(xb[:, :KT // 2, :], xTr[:, :KT // 2, tsl])
                            nc.sync.dma_start(xb[:, KT // 2:, :], xTr[:, KT // 2:, tsl])
                        for m in ([0, 2, 4, 1, 3, 5] if blk == NB - 1 else range(6)):
                            psqk = ps.tile([128, 1024], F32, tag="ps", name="ps")
                            for kt in range(KT):
                                for mh in range(2):
                                    nc.tensor.matmul(
                                        psqk[:, bass.ts(mh, 512)],
                                        wqkv_sb[:, kt, bass.ts(m, 128)],
                                        xb[:, kt, bass.ts(mh, 512)],
                                        start=(kt == 0),
                                        stop=(kt == KT - 1),
                                        skip_group_check=True,
                                    )
                            if m < 4:
                                # q/k: evict with bias, RoPE in place
                                nc.scalar.activation(
                                    qk[m][:, tsl], psqk[:], AF.Identity,
                                    bias=bqkv[:, m:m + 1], scale=1.0,
                                )
                                qm = qk[m][:, tsl]
                                qsw = tp.tile([128, 1024], BF16, tag="qsw", name="qsw")
                                nc.sync.dma_start(qsw[0:64, :], qm[64:128, :])
                                nc.sync.dma_start(qsw[64:128, :], qm[0:64, :])
                                nc.vector.tensor_mul(qsw[:], qsw[:], sin_sb[:, tsl])
                                nc.vector.tensor_mul(qm, qm, cos_sb[:, tsl])
                                nc.vector.tensor_add(qm, qm, qsw[:])
                            else:
                                # v: evict with bias feature-major, then
                                # transpose 128x128 token tiles into v_sb
                                hh = m - 4
                                vT = tp.tile([128, 1024], BF16, tag="qsw", name="vT")
                                nc.scalar.activation(
                                    vT[:], psqk[:], AF.Identity,
                                    bias=bqkv[:, m:m + 1], scale=1.0,
                                )
                                for half in range(2):
                                    pt4 = pst.tile([128, 512], BF16, tag="pt", name="pt")
                                    for tt in range(4):
                                        nc.tensor.transpose(
                                            pt4[:, bass.ts(tt, 128)],
                                            vT[:, bass.ds(half * 512 + tt * 128, 128)],
                                            ident[:],
                                        )
                                    g0 = blk * 8 + half * 4
                                    nc.vector.tensor_copy(
                                        v_sb[:, g0:g0 + 4, bass.ts(hh, HD)],
                                        pt4[:].rearrange("p (a b) -> p a b", a=4),
                                    )

                zg_sb = [pp.tile([128, N_CORES, S], BF16, tag=f"zg{i}", name=f"zg{i}")
                         for i in range(HPC)]

                # out-proj weights live in SBUF freed by the phase-1 pools;
                # first half prefetches on the idle Pool DMA queue during
                # attention, second half after the first AllToAll
                owp_cm = tc.tile_pool(name=f"ow{_it}", bufs=1)
                owp = owp_cm.__enter__()
                ows = [owp.tile([128, KT // 2, D], BF16, tag=f"ow{pa}", name=f"ow{pa}")
                       for pa in range(2)]
                ow_sel = owr[:, 0::2, :]
                for c4 in range(8):
                    nc.gpsimd.dma_start(
                        ows[0][:, c4, :], ow_sel[:, c4, :])

                # ---- phase 2: attention per (head, batch, 512-row block) ----
                # 1024-wide score chunks (2 key-tiles per exp) amortize ACT
                # overhead; scores/exp/PV/denominator are clipped at the
                # causal diagonal (only queries >= 128*m computed for
                # diagonal key-tile m). The softmax denominator accumulates
                # on the PE via ones-matmuls whose M=128 output arrives
                # partition-broadcast, so no broadcast op is needed before
                # the reciprocal. Keeping the denominator on the PE keeps
                # the PE the pacing engine (no cross-engine stalls, pstate
                # stays at full clock).
                with tc.tile_pool(name=f"p2t{_it}", bufs=6) as tp2, \
                     tc.tile_pool(name=f"p2o{_it}", bufs=2) as op2, \
                     tc.tile_pool(name=f"p2r{_it}", bufs=2) as rp2, \
                     tc.tile_pool(name=f"p2ps{_it}", bufs=3, space="PSUM") as ps2, \
                     tc.tile_pool(name=f"p2po{_it}", bufs=1, space="PSUM") as ps2o, \
                     tc.tile_pool(name=f"p2pd{_it}", bufs=1, space="PSUM") as ps2d:
                    for hh in range(HPC):
                        for b in range(B):
                            qT = qk[hh]
                            kTt = qk[2 + hh]
                            for rb in range(T // 512):
                                qbase = b * T + rb * 512
                                pso = ps2o.tile([128, 512], F32, tag="pso", name="pso")
                                psd = ps2d.tile([128, 512], F32, tag="psd", name="psd")
                                njt = 4 * rb + 4
                                for jc in range(njt // 2):
                                    pss = ps2.tile([128, 1024], F32, tag="pss", name="pss")
                                    offs = []
                                    for half in range(2):
                                        jt = 2 * jc + half
                                        m = jt - 4 * rb
                                        off = 128 * m if m >= 0 else 0
                                        offs.append((jt, off))
                                        nc.tensor.matmul(
                                            pss[:, bass.ds(512 * half + off, 512 - off)],
                                            kTt[:, bass.ds(b * T + jt * 128, 128)],
                                            qT[:, bass.ds(qbase + off, 512 - off)],
                                            start=True, stop=True,
                                        )
                                    pT = tp2.tile([128, 1024], BF16, tag="pT", name="pT")
                                    if offs[1][1] == 0:
                                        nc.scalar.activation(
                                            pT[:], pss[:], AF.Exp, scale=SCALE)
                                    else:
                                        for half in range(2):
                                            jt, off = offs[half]
                                            sl = bass.ds(512 * half + off, 512 - off)
                                            nc.scalar.activation(
                                                pT[:, sl], pss[:, sl], AF.Exp,
                                                scale=SCALE)
                                            if jt >= 4 * rb:
                                                tri = bass.ds(512 * half + off, 128)
                                                nc.vector.tensor_mul(
                                                    pT[:, tri], pT[:, tri], masks[:])
                                    for half in range(2):
                                        jt, off = offs[half]
                                        psl = bass.ds(512 * half + off, 512 - off)
                                        osl = bass.ds(off, 512 - off)
                                        nc.tensor.matmul(
                                            psd[:, osl], onesB[:], pT[:, psl],
                                            start=(jt == 0), stop=(jt == njt - 1),
                                            skip_group_check=True,
                                        )
                                        nc.tensor.matmul(
                                            pso[:, osl],
                                            v_sb[:, (b * T) // 128 + jt, bass.ts(hh, HD)],
                                            pT[:, psl],
                                            start=(jt == 0),
                                            stop=(jt == njt - 1),
                                            skip_group_check=True,
                                        )
                                recipB = rp2.tile([128, 512], F32, tag="recipB",
                                                  name="recipB")
                                nc.vector.reciprocal(recipB[:], psd[:])
                                oT = op2.tile([128, 512], BF16, tag="oT", name="oT")
                                nc.vector.tensor_mul(oT[:], pso[:], recipB[:])
                                sh = qbase // S
                                nc.sync.dma_start(Zs[hh][sh, :, :], oT[:])
                        # reshard this head-half while the next one computes
                        if fake_cc:
                            nc.sync.dma_start(ZGs[hh][:], Zs[hh][:])
                        else:
                            nc.gpsimd.collective_compute(
                                "AllToAll", ALU.bypass,
                                replica_groups=[list(range(N_CORES))],
                                ins=[Zs[hh][:]], outs=[ZGs[hh][:]],
                            )
                        # pull this half's out-proj operand into SBUF on the
                        # Pool queue (so its collective-wait never blocks the
                        # SP store queue), chunked so out-proj can start on
                        # the first chunk
                        for c8 in range(N_CORES):
                            nc.gpsimd.dma_start(
                                zg_sb[hh][:, c8, :], ZGs[hh][c8, :, :])
                        if hh == 0:
                            # prefetch second half of out-proj weights now
                            ow_sel = owr[:, 1::2, :]
                            for c4 in range(8):
                                nc.gpsimd.dma_start(
                                    ows[1][:, c4, :], ow_sel[:, c4, :])

            # ---- phase 4: out projection on own token slice ----
            # Two-pass contraction: all zg0 (first AllToAll) partial sums are
            # computed and evicted before any zg1 tile is touched, so the
            # second AllToAll and the zg1 load hide behind real matmul work.
            with tc.tile_pool(name=f"p4z{_it}", bufs=1) as zp, \
                 tc.tile_pool(name=f"p4t{_it}", bufs=4) as tp4, \
                 tc.tile_pool(name=f"p4ps{_it}", bufs=2, space="PSUM") as ps4:
                accbig = zp.tile([128, S // 128, D], F32, tag="accbig", name="accbig")
                # keep the PE busy (clock ramped) while the first zg chunks
                # arrive: harmless matmuls on resident attention tiles
                psj = ps4.tile([128, D], F32, tag="ps4", name="psj")
                for ju in range(12):
                    nc.tensor.matmul(
                        psj[:, bass.ts(ju % 4, 512)],
                        qk[0][:, bass.ts(ju, 128)], qk[2][:, bass.ts(0, 512)],
                        start=True, stop=True, skip_group_check=True,
                    )
                for pa in range(2):
                    if pa == 1:
                        # filler while the second AllToAll lands
                        psj2 = ps4.tile([128, D], F32, tag="ps4", name="psj2")
                        for ju in range(40):
                            nc.tensor.matmul(
                                psj2[:, bass.ts(ju % 4, 512)],
                                zg_sb[0][:, ju % 8, bass.ts(0, 128)],
                                ows[0][:, ju % 8, bass.ts(ju % 4, 512)],
                                start=True, stop=True, skip_group_check=True,
                            )
                    for tt in range(S // 128):
                        pso4 = ps4.tile([128, D], F32, tag="ps4", name="ps4")
                        for zt in range(KT // HPC):
                            for eh in range(4):
                                nc.tensor.matmul(
                                    pso4[:, bass.ts(eh, 512)],
                                    zg_sb[pa][:, zt, bass.ts(tt, 128)],
                                    ows[pa][:, zt, bass.ts(eh, 512)],
                                    start=(zt == 0),
                                    stop=(zt == KT // HPC - 1),
                                    skip_group_check=True,
                                )
                        if pa == 0:
                            # fold the output bias in here so pass 1 needs a
                            # single add before each store
                            nc.vector.tensor_add(accbig[:, tt, :], pso4[:], obB[:])
                        else:
                            # finer-grained eviction+store pipeline to shrink
                            # the end-of-kernel DMA tail
                            for e in range(4):
                                esl = bass.ts(e, 512)
                                of = tp4.tile([128, 512], F32, tag="of", name="of")
                                nc.vector.tensor_add(
                                    of[:], pso4[:, esl], accbig[:, tt, esl])
                                nc.sync.dma_start(out_e[bass.ts(tt, 128), esl], of[:])
            owp_cm.__exit__(None, None, None)

    nc.compile()          # Bacc pass pipeline (library loads, nop fusion, regs)
    legalize_waits(nc)    # must run after all nop-fusion passes
    bass.Bass.finalize(nc)  # freeze without re-running Bacc compile
    return nc


def _prep_inputs(x, rope_cos, rope_sin, qkv_w, qkv_b, out_w, out_b, B, T, D, H):
    HD = D // H
    NT = B * T
    HPC = H // N_CORES
    bf = ml_dtypes.bfloat16

    x2 = np.ascontiguousarray(x.reshape(NT, D).T).astype(bf)           # [D, NT]
    cosT = np.ascontiguousarray(
        np.tile(rope_cos[0, 0].T, (1, B))).astype(bf)                   # [HD, NT]
    s2 = np.tile(rope_sin[0, 0].T, (1, B)).copy()
    s2[:HD // 2] *= -1.0
    sinT = np.ascontiguousarray(s2).astype(bf)
    owT = np.ascontiguousarray(out_w.T).astype(bf)                      # [D, D]
    ob = out_b.astype(np.float32)

    c_grid = np.arange(128)[None, :]
    p_grid = np.arange(128)[:, None]
    masks = (c_grid >= p_grid).astype(bf)                               # [128,128]
    negm = np.where(c_grid >= p_grid, 0.0, -1e5).astype(np.float32)     # [128,128]
    ident = np.eye(128).astype(bf)

    in_maps = []
    for c in range(N_CORES):
        heads = [HPC * c + i for i in range(HPC)]
        q_rows = np.concatenate([qkv_w[h * HD:(h + 1) * HD] for h in heads])
        k_rows = np.concatenate([qkv_w[D + h * HD:D + (h + 1) * HD] for h in heads])
        v_rows = np.concatenate([qkv_w[2 * D + h * HD:2 * D + (h + 1) * HD] for h in heads])
        wqkv = np.ascontiguousarray(
            np.concatenate([q_rows, k_rows, v_rows]).T).astype(bf)      # [D, 768]
        bq = np.concatenate([qkv_b[h * HD:(h + 1) * HD] for h in heads])
        bk = np.concatenate([qkv_b[D + h * HD:D + (h + 1) * HD] for h in heads])
        bv = np.concatenate([qkv_b[2 * D + h * HD:2 * D + (h + 1) * HD] for h in heads])
        bqkv = np.concatenate([bq, bk, bv]).astype(np.float32)          # [768]
        in_maps.append({
            "xT": x2, "wqkv": wqkv, "bqkv": bqkv,
            "cosT": cosT, "sinT": sinT, "masks": masks, "negm": negm,
            "ident": ident,
            "owT": owT, "ob": ob,
        })
    return in_maps


_NC_CACHE = {}


def kernel(x, rope_cos, rope_sin, qkv_w, qkv_b, out_w, out_b):
    B, T, D = x.shape
    H = 16
    NT = B * T
    S = NT // N_CORES
    key = (B, T, D, H)
    if key not in _NC_CACHE:
        _NC_CACHE[key] = build_nc(B, T, D, H)
    nc = _NC_CACHE[key]
    in_maps = _prep_inputs(
        np.asarray(x), np.asarray(rope_cos), np.asarray(rope_sin),
        np.asarray(qkv_w), np.asarray(qkv_b), np.asarray(out_w),
        np.asarray(out_b), B, T, D, H,
    )
    res = run_bass_kernel_spmd(nc, in_maps, core_ids=list(range(N_CORES)))
    out = np.empty((NT, D), np.float32)
    for c in range(N_CORES):
        out[c * S:(c + 1) * S] = res.results[c]["out"]
    return out.reshape(B, T, D)


# revision 55
# speedup vs baseline: 1.1908x; 1.0527x over previous
"""Causal self-attention with RoPE on 8 Trainium2 NeuronCores.

Sharding: tensor-parallel over heads (2 heads/core) through QKV projection,
RoPE and attention; AllToAll reshards attention output from head-split to
token-split; out-projection is token-parallel with full out_w per core
(no reduction needed). Output: each core produces its 512-token slice.

The kernel is PE-sequencer/instruction-bound (~240 ns issue cost per matmul),
so the design minimizes matmul instruction count and keeps the PE free of
softmax-denominator streaming:
  phase 1: fused q/k/v projection, 6 f-tiles x 16 kt x 4 blocks of N=1024
           (384 matmuls); v is produced feature-major and transposed on the
           PE (64 [128,128] transposes) into token-major v_sb for PV.
  phase 2: per (head, batch, 1024-query half): one scores matmul per
           key-tile (N clipped at the causal diagonal), exp on ACT, one PV
           matmul per key-tile. The softmax denominator is accumulated
           elementwise in bf16: the first 512 queries on the Pool engine,
           the second 512 on DVE (both otherwise idle), then reduced across
           partitions by two ones-matmuls into a PSUM tile borrowed from
           the scores pool ([128,1024] output = partition-broadcast sums).
  phase 4: out-projection with N=2048 (64 matmuls), two passes (even kt from
           the first AllToAll, odd kt from the second) so each AllToAll
           hides behind compute; out-proj weights prefetch on the Pool DMA
           queue during earlier phases.
"""
import math
import numpy as np
import ml_dtypes

import concourse.bass as bass
import concourse.mybir as mybir
import concourse.tile as tile
from concourse import bacc
from concourse.bass_utils import run_bass_kernel_spmd

F32 = mybir.dt.float32
F32R = mybir.dt.float32r
BF16 = mybir.dt.bfloat16
AF = mybir.ActivationFunctionType
ALU = mybir.AluOpType

N_CORES = 8


def legalize_waits(nc, max_waits=1):
    """This walrus build only encodes one sync-wait per TPB instruction.
    Move extra waits emitted by Tile onto same-engine NoOps inserted
    immediately before the instruction."""
    n_split = 0
    for fn in nc.m.functions:
        for bb in fn.blocks:
            new_insts = []
            for inst in bb.instructions:
                si = getattr(inst, "sync_info", None)
                waits = list(si.on_wait) if si is not None and si.on_wait else []
                if len(waits) > max_waits and type(inst).__name__ != "InstNoOp":
                    extra, keep = waits[:-max_waits], waits[-max_waits:]
                    for k, w in enumerate(extra):
                        nop = mybir.InstNoOp(
                            name=f"{inst.name}_waitnop{k}",
                            engine=inst.engine,
                            ins=[],
                            outs=[],
                            sync_info=mybir.SyncInfo(on_wait=[w], on_update=[]),
                        )
                        nc.register_instruction(nop)
                        new_insts.append(nop)
                    inst.sync_info = mybir.SyncInfo(
                        on_wait=keep, on_update=list(si.on_update)
                    )
                    n_split += 1
                new_insts.append(inst)
            bb.instructions = new_insts
    return n_split


def build_nc(B=2, T=2048, D=2048, H=16, fake_cc=False, n_loop=1, dummy_io=False):
    HD = D // H                  # 128, head dim
    NT = B * T                   # total tokens
    HPC = H // N_CORES           # heads per core (2)
    DC = HPC * HD                # head channels per core (256)
    KT = D // 128                # contraction tiles for projections (16)
    NB = NT // 1024              # 1024-token projection blocks (4)
    GH = T // 1024               # 1024-query halves per batch element (2)
    S = NT // N_CORES            # AllToAll shard = tokens per core (512)
    SCALE = 1.0 / math.sqrt(HD)

    nc = bacc.Bacc("TRN2", target_bir_lowering=False, debug=False, num_devices=N_CORES)
    # dummy_io: declare data tensors as internal DRAM (uninitialized) so the
    # timing NEFF has no big inputs to ship through the axon tunnel.
    ik = {"kind": "ExternalInput"} if not dummy_io else {}
    xT_e = nc.dram_tensor("xT", [D, NT], BF16, **ik)
    wqkv_e = nc.dram_tensor("wqkv", [D, 6 * HD], BF16, **ik)
    bqkv_e = nc.dram_tensor("bqkv", [6 * HD], F32, **ik)
    cos_e = nc.dram_tensor("cosT", [HD, NT], BF16, **ik)
    sin_e = nc.dram_tensor("sinT", [HD, NT], BF16, **ik)
    masks_e = nc.dram_tensor("masks", [128, 128], BF16, **ik)
    negm_e = nc.dram_tensor("negm", [128, 128], F32, **ik)
    ident_e = nc.dram_tensor("ident", [128, 128], BF16, **ik)
    owT_e = nc.dram_tensor("owT", [D, D], BF16, **ik)
    ob_e = nc.dram_tensor("ob", [D], F32, **ik)
    out_e = nc.dram_tensor("out", [S, D], F32, kind="ExternalOutput")

    with tile.TileContext(nc) as tc:
      for _it in range(n_loop):
        with tc.tile_pool(name=f"persist{_it}", bufs=1) as pp, \
             tc.tile_pool(name=f"dram{_it}", bufs=1, space="DRAM") as dp:
            owr = owT_e.rearrange("(kt p) f -> p kt f", p=128)

            # ---- persistent small tiles ----
            masks = pp.tile([128, 128], BF16, tag="masks", name="masks")
            nc.sync.dma_start(masks[:], masks_e[:])
            negm = pp.tile([128, 128], F32, tag="negm", name="negm")
            nc.sync.dma_start(negm[:], negm_e[:])
            ident = pp.tile([128, 128], BF16, tag="ident", name="ident")
            nc.sync.dma_start(ident[:], ident_e[:])
            bqkv = pp.tile([128, 6], F32, tag="bqkv", name="bqkv")
            nc.sync.dma_start(bqkv[:], bqkv_e.rearrange("(m p) -> p m", p=128))
            obB = pp.tile([128, D], F32, tag="obB", name="obB")
            nc.sync.dma_start(obB[:1, :], ob_e[None, :])
            nc.gpsimd.partition_broadcast(obB[:], obB[:1, :])
            onesB = pp.tile([128, 128], BF16, tag="onesB", name="onesB")
            nc.vector.memset(onesB[:], 1.0)

            Zs = [dp.tile([N_CORES, HD, S], BF16, tag=f"Z{i}", name=f"Z{i}")
                  for i in range(HPC)]
            ZGs = [dp.tile([N_CORES, HD, S], BF16, tag=f"ZG{i}", name=f"ZG{i}")
                   for i in range(HPC)]

            if True:
                # q/k feature-major [head-dim, tokens]; v token-major
                qk = [pp.tile([128, NT], BF16, tag=f"qk{m}", name=f"qk{m}")
                      for m in range(4)]
                v_sb = pp.tile([128, NT // 128, DC], BF16, tag="v", name="v")

                # ---- phase 1: fused QKV projection + RoPE + v transpose ----
                with tc.tile_pool(name=f"p1w{_it}", bufs=1) as wp, \
                     tc.tile_pool(name=f"p1x{_it}", bufs=2) as xp, \
                     tc.tile_pool(name=f"p1t{_it}", bufs=4) as tp, \
                     tc.tile_pool(name=f"p1ps{_it}", bufs=3, space="PSUM") as ps, \
                     tc.tile_pool(name=f"p1pt{_it}", bufs=2, space="PSUM") as pst:
                    # PE warm-up: junk transposes keep the PE busy (and the
                    # clock ramped) while the first x/weight DMAs land
                    for wu in range(144):
                        ptw = pst.tile([128, 512], BF16, tag="pt", name="ptw")
                        nc.tensor.transpose(
                            ptw[:, bass.ts(wu % 4, 128)], onesB[:], onesB[:])
                    # first-needed data first: x block 0, then qkv weights
                    xTr = xT_e.rearrange("(kt p) t -> p kt t", p=128)
                    xb0 = xp.tile([128, KT, 1024], BF16, tag="xb", name="xb")
                    nc.sync.dma_start(xb0[:, :KT // 2, :], xTr[:, :KT // 2, bass.ts(0, 1024)])
                    nc.sync.dma_start(xb0[:, KT // 2:, :], xTr[:, KT // 2:, bass.ts(0, 1024)])
                    wqkv_sb = wp.tile([128, KT, 6 * HD], BF16, tag="wqkv", name="wqkv")
                    wqr = wqkv_e.rearrange("(kt p) f -> p kt f", p=128)
                    nc.scalar.dma_start(wqkv_sb[:, :KT // 2, :], wqr[:, :KT // 2, :])
                    nc.scalar.dma_start(wqkv_sb[:, KT // 2:, :], wqr[:, KT // 2:, :])
                    cos_sb = wp.tile([128, NT], BF16, tag="cos", name="cos")
                    nc.scalar.dma_start(cos_sb[:], cos_e[:])
                    sin_sb = wp.tile([128, NT], BF16, tag="sin", name="sin")
                    nc.scalar.dma_start(sin_sb[:], sin_e[:])
                    for blk in range(NB):
                        tsl = bass.ts(blk, 1024)
                        if blk == 0:
                            xb = xb0
                        else:
                            xb = xp.tile([128, KT, 1024], BF16, tag="xb", name="xb")
                            nc.sync.dma_start(xb[:, :KT // 2, :], xTr[:, :KT // 2, tsl])
                            nc.sync.dma_start(xb[:, KT // 2:, :], xTr[:, KT // 2:, tsl])
                        for m in ([0, 2, 4, 1, 3, 5] if blk == NB - 1 else range(6)):
                            psqk = ps.tile([128, 1024], F32, tag="ps", name="ps")
                            for kt in range(KT):
                                for mh in range(2):
                                    nc.tensor.matmul(
                                        psqk[:, bass.ts(mh, 512)],
                                        wqkv_sb[:, kt, bass.ts(m, 128)],
                                        xb[:, kt, bass.ts(mh, 512)],
                                        start=(kt == 0),
                                        stop=(kt == KT - 1),
                                        skip_group_check=True,
                                    )
                            if m < 4:
                                # q/k: evict with bias, RoPE in place
                                nc.scalar.activation(
                                    qk[m][:, tsl], psqk[:], AF.Identity,
                                    bias=bqkv[:, m:m + 1], scale=1.0,
                                )
                                qm = qk[m][:, tsl]
                                qsw = tp.tile([128, 1024], BF16, tag="qsw", name="qsw")
                                nc.sync.dma_start(qsw[0:64, :], qm[64:128, :])
                                nc.sync.dma_start(qsw[64:128, :], qm[0:64, :])
                                nc.vector.tensor_mul(qsw[:], qsw[:], sin_sb[:, tsl])
                                nc.vector.tensor_mul(qm, qm, cos_sb[:, tsl])
                                nc.vector.tensor_add(qm, qm, qsw[:])
                            else:
                                # v: evict with bias feature-major, then
                                # transpose 128x128 token tiles into v_sb
                                hh = m - 4
                                vT = tp.tile([128, 1024], BF16, tag="qsw", name="vT")
                                nc.scalar.activation(
                                    vT[:], psqk[:], AF.Identity,
                                    bias=bqkv[:, m:m + 1], scale=1.0,
                                )
                                for half in range(2):
                                    pt4 = pst.tile([128, 512], BF16, tag="pt", name="pt")
                                    for tt in range(4):
                                        nc.tensor.transpose(
                                            pt4[:, bass.ts(tt, 128)],
                                            vT[:, bass.ds(half * 512 + tt * 128, 128)],
                                            ident[:],
                                        )
                                    g0 = blk * 8 + half * 4
                                    nc.vector.tensor_copy(
                                        v_sb[:, g0:g0 + 4, bass.ts(hh, HD)],
                                        pt4[:].rearrange("p (a b) -> p a b", a=4),
                                    )

                zg_sb = [pp.tile([128, N_CORES, S], BF16, tag=f"zg{i}", name=f"zg{i}")
                         for i in range(HPC)]

                # out-proj weights live in SBUF freed by the phase-1 pools;
                # first half prefetches on the idle Pool DMA queue during
                # attention, second half after the first AllToAll
                owp_cm = tc.tile_pool(name=f"ow{_it}", bufs=1)
                owp = owp_cm.__enter__()
                ows = [owp.tile([128, KT // 2, D], BF16, tag=f"ow{pa}", name=f"ow{pa}")
                       for pa in range(2)]
                ow_sel = owr[:, 0::2, :]
                for c4 in range(8):
                    nc.gpsimd.dma_start(
                        ows[0][:, c4, :], ow_sel[:, c4, :])

                # ---- phase 2: attention per (head, batch, 512-row block) ----
                # 1024-wide score chunks (2 key-tiles per exp) amortize ACT
                # overhead; scores/exp/PV/denominator are clipped at the
                # causal diagonal (only queries >= 128*m computed for
                # diagonal key-tile m). The softmax denominator accumulates
                # on the PE via ones-matmuls whose M=128 output arrives
                # partition-broadcast, so no broadcast op is needed before
                # the reciprocal. Keeping the denominator on the PE keeps
                # the PE the pacing engine (no cross-engine stalls, pstate
                # stays at full clock).
                with tc.tile_pool(name=f"p2t{_it}", bufs=6) as tp2, \
                     tc.tile_pool(name=f"p2o{_it}", bufs=2) as op2, \
                     tc.tile_pool(name=f"p2r{_it}", bufs=2) as rp2, \
                     tc.tile_pool(name=f"p2ps{_it}", bufs=3, space="PSUM") as ps2, \
                     tc.tile_pool(name=f"p2po{_it}", bufs=1, space="PSUM") as ps2o, \
                     tc.tile_pool(name=f"p2pd{_it}", bufs=1, space="PSUM") as ps2d:
                    for hh in range(HPC):
                        for b in range(B):
                            qT = qk[hh]
                            kTt = qk[2 + hh]
                            for rb in range(T // 512):
                                qbase = b * T + rb * 512
                                pso = ps2o.tile([128, 512], F32, tag="pso", name="pso")
                                psd = ps2d.tile([128, 512], F32, tag="psd", name="psd")
                                njt = 4 * rb + 4
                                for jc in range(njt // 2):
                                    pss = ps2.tile([128, 1024], F32, tag="pss", name="pss")
                                    offs = []
                                    for half in range(2):
                                        jt = 2 * jc + half
                                        m = jt - 4 * rb
                                        off = 128 * m if m >= 0 else 0
                                        offs.append((jt, off))
                                        nc.tensor.matmul(
                                            pss[:, bass.ds(512 * half + off, 512 - off)],
                                            kTt[:, bass.ds(b * T + jt * 128, 128)],
                                            qT[:, bass.ds(qbase + off, 512 - off)],
                                            start=True, stop=True,
                                        )
                                    pT = tp2.tile([128, 1024], BF16, tag="pT", name="pT")
                                    if offs[1][1] == 0:
                                        nc.scalar.activation(
                                            pT[:], pss[:], AF.Exp, scale=SCALE)
                                    else:
                                        for half in range(2):
                                            jt, off = offs[half]
                                            sl = bass.ds(512 * half + off, 512 - off)
                                            nc.scalar.activation(
                                                pT[:, sl], pss[:, sl], AF.Exp,
                                                scale=SCALE)
                                            if jt >= 4 * rb:
                                                tri = bass.ds(512 * half + off, 128)
                                                nc.vector.tensor_mul(
                                                    pT[:, tri], pT[:, tri], masks[:])
                                    for half in range(2):
                                        jt, off = offs[half]
                                        psl = bass.ds(512 * half + off, 512 - off)
                                        osl = bass.ds(off, 512 - off)
                                        nc.tensor.matmul(
                                            psd[:, osl], onesB[:], pT[:, psl],
                                            start=(jt == 0), stop=(jt == njt - 1),
                                            skip_group_check=True,
                                        )
                                        nc.tensor.matmul(
                                            pso[:, osl],
                                            v_sb[:, (b * T) // 128 + jt, bass.ts(hh, HD)],
                                            pT[:, psl],
                                            start=(jt == 0),
                                            stop=(jt == njt - 1),
                                            skip_group_check=True,
                                        )
                                recipB = rp2.tile([128, 512], F32, tag="recipB",
                                                  name="recipB")
                                nc.vector.reciprocal(recipB[:], psd[:])
                                oT = op2.tile([128, 512], BF16, tag="oT", name="oT")
                                nc.vector.tensor_mul(oT[:], pso[:], recipB[:])
                                sh = qbase // S
                                nc.sync.dma_start(Zs[hh][sh, :, :], oT[:])
                        # reshard this head-half while the next one computes
                        if fake_cc:
                            nc.sync.dma_start(ZGs[hh][:], Zs[hh][:])
                        else:
                            nc.gpsimd.collective_compute(
                                "AllToAll", ALU.bypass,
                                replica_groups=[list(range(N_CORES))],
                                ins=[Zs[hh][:]], outs=[ZGs[hh][:]],
                            )
                        # pull this half's out-proj operand into SBUF on the
                        # Pool queue (so its collective-wait never blocks the
                        # SP store queue), chunked so out-proj can start on
                        # the first chunk
                        for c8 in range(N_CORES):
                            nc.gpsimd.dma_start(
                                zg_sb[hh][:, c8, :], ZGs[hh][c8, :, :])
                        if hh == 0:
                            # prefetch second half of out-proj weights now
                            ow_sel = owr[:, 1::2, :]
                            for c4 in range(8):
                                nc.gpsimd.dma_start(
                                    ows[1][:, c4, :], ow_sel[:, c4, :])

            # ---- phase 4: out projection on own token slice ----
            # Two-pass contraction: all zg0 (first AllToAll) partial sums are
            # computed and evicted before any zg1 tile is touched, so the
            # second AllToAll and the zg1 load hide behind real matmul work.
            with tc.tile_pool(name=f"p4z{_it}", bufs=1) as zp, \
                 tc.tile_pool(name=f"p4t{_it}", bufs=4) as tp4, \
                 tc.tile_pool(name=f"p4ps{_it}", bufs=2, space="PSUM") as ps4:
                accbig = zp.tile([128, S // 128, D], F32, tag="accbig", name="accbig")
                # keep the PE busy (clock ramped) while the first zg chunks
                # arrive: harmless matmuls on resident attention tiles
                psj = ps4.tile([128, D], F32, tag="ps4", name="psj")
                for ju in range(12):
                    nc.tensor.matmul(
                        psj[:, bass.ts(ju % 4, 512)],
                        qk[0][:, bass.ts(ju, 128)], qk[2][:, bass.ts(0, 512)],
                        start=True, stop=True, skip_group_check=True,
                    )
                for pa in range(2):
                    if pa == 1:
                        # filler while the second AllToAll lands
                        psj2 = ps4.tile([128, D], F32, tag="ps4", name="psj2")
                        for ju in range(40):
                            nc.tensor.matmul(
                                psj2[:, bass.ts(ju % 4, 512)],
                                zg_sb[0][:, ju % 8, bass.ts(0, 128)],
                                ows[0][:, ju % 8, bass.ts(ju % 4, 512)],
                                start=True, stop=True, skip_group_check=True,
                            )
                    for tt in range(S // 128):
                        pso4 = ps4.tile([128, D], F32, tag="ps4", name="ps4")
                        for zt in range(KT // HPC):
                            for eh in range(4):
                                nc.tensor.matmul(
                                    pso4[:, bass.ts(eh, 512)],
                                    zg_sb[pa][:, zt, bass.ts(tt, 128)],
                                    ows[pa][:, zt, bass.ts(eh, 512)],
                                    start=(zt == 0),
                                    stop=(zt == KT // HPC - 1),
                                    skip_group_check=True,
                                )
                        if pa == 0:
                            # fold the output bias in here so pass 1 needs a
                            # single add before each store
                            nc.vector.tensor_add(accbig[:, tt, :], pso4[:], obB[:])
                        else:
                            # finer-grained eviction+store pipeline to shrink
                            # the end-of-kernel DMA tail
                            for e in range(4):
                                esl = bass.ts(e, 512)
                                of = tp4.tile([128, 512], F32, tag="of", name="of")
                                nc.vector.tensor_add(
                                    of[:], pso4[:, esl], accbig[:, tt, esl])
                                nc.sync.dma_start(out_e[bass.ts(tt, 128), esl], of[:])
            owp_cm.__exit__(None, None, None)

    nc.compile()          # Bacc pass pipeline (library loads, nop fusion, regs)
    legalize_waits(nc)    # must run after all nop-fusion passes
    bass.Bass.finalize(nc)  # freeze without re-running Bacc compile
    return nc


def _prep_inputs(x, rope_cos, rope_sin, qkv_w, qkv_b, out_w, out_b, B, T, D, H):
    HD = D // H
    NT = B * T
    HPC = H // N_CORES
    bf = ml_dtypes.bfloat16

    x2 = np.ascontiguousarray(x.reshape(NT, D).T).astype(bf)           # [D, NT]
    cosT = np.ascontiguousarray(
        np.tile(rope_cos[0, 0].T, (1, B))).astype(bf)                   # [HD, NT]
    s2 = np.tile(rope_sin[0, 0].T, (1, B)).copy()
    s2[:HD // 2] *= -1.0
    sinT = np.ascontiguousarray(s2).astype(bf)
    owT = np.ascontiguousarray(out_w.T).astype(bf)                      # [D, D]
    ob = out_b.astype(np.float32)

    c_grid = np.arange(128)[None, :]
    p_grid = np.arange(128)[:, None]
    masks = (c_grid >= p_grid).astype(bf)                               # [128,128]
    negm = np.where(c_grid >= p_grid, 0.0, -1e5).astype(np.float32)     # [128,128]
    ident = np.eye(128).astype(bf)

    in_maps = []
    for c in range(N_CORES):
        heads = [HPC * c + i for i in range(HPC)]
        q_rows = np.concatenate([qkv_w[h * HD:(h + 1) * HD] for h in heads])
        k_rows = np.concatenate([qkv_w[D + h * HD:D + (h + 1) * HD] for h in heads])
        v_rows = np.concatenate([qkv_w[2 * D + h * HD:2 * D + (h + 1) * HD] for h in heads])
        wqkv = np.ascontiguousarray(
            np.concatenate([q_rows, k_rows, v_rows]).T).astype(bf)      # [D, 768]
        bq = np.concatenate([qkv_b[h * HD:(h + 1) * HD] for h in heads])
        bk = np.concatenate([qkv_b[D + h * HD:D + (h + 1) * HD] for h in heads])
        bv = np.concatenate([qkv_b[2 * D + h * HD:2 * D + (h + 1) * HD] for h in heads])
        bqkv = np.concatenate([bq, bk, bv]).astype(np.float32)          # [768]
        in_maps.append({
            "xT": x2, "wqkv": wqkv, "bqkv": bqkv,
            "cosT": cosT, "sinT": sinT, "masks": masks, "negm": negm,
            "ident": ident,
            "owT": owT, "ob": ob,
        })
    return in_maps


_NC_CACHE = {}


def kernel(x, rope_cos, rope_sin, qkv_w, qkv_b, out_w, out_b):
    B, T, D = x.shape
    H = 16
    NT = B * T
    S = NT // N_CORES
    key = (B, T, D, H)
    if key not in _NC_CACHE:
        _NC_CACHE[key] = build_nc(B, T, D, H)
    nc = _NC_CACHE[key]
    in_maps = _prep_inputs(
        np.asarray(x), np.asarray(rope_cos), np.asarray(rope_sin),
        np.asarray(qkv_w), np.asarray(qkv_b), np.asarray(out_w),
        np.asarray(out_b), B, T, D, H,
    )
    res = run_bass_kernel_spmd(nc, in_maps, core_ids=list(range(N_CORES)))
    out = np.empty((NT, D), np.float32)
    for c in range(N_CORES):
        out[c * S:(c + 1) * S] = res.results[c]["out"]
    return out.reshape(B, T, D)
